# revision 22
# baseline (speedup 1.0000x reference)
"""CrossAttention2D Trainium2 kernel (bf16 compute).

Sharding: data-parallel over batch. B=8 -> one batch element per NeuronCore,
no collectives. Weights replicated; host pre-transposes and casts to bf16.

Per-core math (C=512, Ccross=768, N=1024, 8 heads x 64):
  Q = Wq @ x_b          [C, N]   (lhsT = WqT tiles, bf16)
  K = Wk @ y_b          [C, N]
  VT = (Wv @ y_b).T     [N, C]   (lhsT = y tiles, rhs = WvT; bias via K=1 matmul)
  per head pair ph (heads on PE rows 0-63 / 64-127, row-tiled concurrent MMs):
    ST = K_h^T . Q_h              [k, q] tiles (K=64 matmuls)
    ET = exp(0.125 * ST)          (ScalarE, free=1024 per instr, PSUM -> SBUF bf16)
    OT_aug = [VT_h | 1].T @ ET    [65, q] in [65,512] accumulation groups
    PE-transpose -> [q, 65]; DVE: O = OT[:, :64] * (1/OT[:, 64])
  quirk: out_flat[h*64+r, j*64+d] = O_h[16r+j, d]  (SBUF->SBUF DMAs)
  out = Wo @ quirk + bo  [C, N] fp32

Schedule: the scores+exp loop is ScalarE-bound; AV bursts + transposes +
tail of the previous pair and projections of the next pair are interleaved
into the PE stream to keep the HAM clock-gate warm. A dummy-matmul warmup
runs during the input-DMA head. PSUM: psS 2x[128,1024] (4 banks) +
psX 4x[128,512]-class shared by AV/transpose/proj (4 banks) = 8.
"""

import numpy as np

import concourse.bass as bass
import concourse.mybir as mybir
import concourse.tile as tile
from concourse import bacc
from concourse.bass_utils import run_bass_kernel_spmd
from concourse.masks import make_identity

P = 128
C = 512          # d_embed
CC = 768         # d_cross
N = 1024         # H*W = 32*32
NH = 8
DH = 64
CT = C // P      # 4
CCT = CC // P    # 6
QT = N // P      # 8
HW = 32
B = 8
F32 = mybir.dt.float32
BF16 = mybir.dt.bfloat16

_CACHE = {}


def _build_nc():
    nc = bacc.Bacc("TRN2", target_bir_lowering=False, debug=False, num_devices=B)

    x = nc.dram_tensor("x", [C, N], BF16, kind="ExternalInput")
    y = nc.dram_tensor("y", [CC, N], BF16, kind="ExternalInput")
    wqT = nc.dram_tensor("wqT", [C, C], BF16, kind="ExternalInput")
    wkT = nc.dram_tensor("wkT", [CC, C], BF16, kind="ExternalInput")
    wvT = nc.dram_tensor("wvT", [CC, C], BF16, kind="ExternalInput")
    woT = nc.dram_tensor("woT", [C, C], BF16, kind="ExternalInput")
    bq = nc.dram_tensor("bq", [C], F32, kind="ExternalInput")
    bk = nc.dram_tensor("bk", [C], F32, kind="ExternalInput")
    bv = nc.dram_tensor("bv", [C], BF16, kind="ExternalInput")
    bo = nc.dram_tensor("bo", [C], F32, kind="ExternalInput")
    out = nc.dram_tensor("out", [C, N], F32, kind="ExternalOutput")

    EXP = mybir.ActivationFunctionType.Exp

    with tile.TileContext(nc) as tc:
        with (
            tc.tile_pool(name="const", bufs=1) as constp,
            tc.tile_pool(name="big", bufs=1) as bigp,
            tc.tile_pool(name="et", bufs=1) as etp,
            tc.tile_pool(name="ot", bufs=4) as otp,
            tc.tile_pool(name="rcp", bufs=4) as rcpp,
            tc.tile_pool(name="ev", bufs=4) as evp,
            tc.tile_pool(name="psS", bufs=2, space="PSUM") as psS,
            tc.tile_pool(name="psP", bufs=2, space="PSUM") as psP,
            tc.tile_pool(name="psX", bufs=2, space="PSUM") as psX,
        ):
            # ---- constants ----
            junk_bf = constp.tile([P, P], BF16, name="junk", tag="junk")
            nc.gpsimd.memset(junk_bf[:], 0.125)
            ident = constp.tile([P, P], F32, name="ident", tag="ident")
            make_identity(nc, ident)
            ones_bf = constp.tile([1, P], BF16, name="ones_bf", tag="ones_bf")
            nc.vector.tensor_scalar(
                ones_bf[:], ident[0:1, :], 0.0, 1.0,
                mybir.AluOpType.mult, mybir.AluOpType.add,
            )
            bq_sb = constp.tile([P, CT], F32, name="bq", tag="bq")
            bk_sb = constp.tile([P, CT], F32, name="bk", tag="bk")
            bo_sb = constp.tile([P, CT], F32, name="bo", tag="bo")
            bv_sb = constp.tile([1, C], BF16, name="bv", tag="bv")

            # ---- PE + ACT warmup (runs while input DMAs land) ----
            # ~36 dummy matmuls keep the PE busy >3.4us so the HAM clock
            # gate reaches 8/8 before the first real projection; one junk
            # exp pulls the ACT table load off the critical path.
            psw = psP.tile([P, P], F32, name="psw", tag="psp")
            for _ in range(40):
                nc.tensor.matmul(psw[:], junk_bf[:], junk_bf[:],
                                 start=True, stop=True)

            def pad(n):
                # dependency-free LDWEIGHTS keep the PE HAM activity monitor
                # busy through ScalarE-bound stretches (junk loads are safe:
                # every real matmul self-loads its own weights)
                for _ in range(n):
                    nc.tensor.ldweights(junk_bf[:])

            # ---- weight / activation loads (per-ktile DMAs for queue spread) ----
            x3 = x.rearrange("(t p) n -> p t n", p=P)
            y3 = y.rearrange("(t p) n -> p t n", p=P)
            wq3 = wqT.rearrange("(t p) m -> p t m", p=P)
            wk3 = wkT.rearrange("(t p) m -> p t m", p=P)
            wv3 = wvT.rearrange("(t p) m -> p t m", p=P)
            wo3 = woT.rearrange("(t p) m -> p t m", p=P)

            x_sb = [bigp.tile([P, N], BF16, name=f"x{t}", tag=f"x{t}") for t in range(CT)]
            y_sb = [bigp.tile([P, N], BF16, name=f"y{t}", tag=f"y{t}") for t in range(CCT)]
            wq_sb = [bigp.tile([P, C], BF16, name=f"wq{t}", tag=f"wq{t}") for t in range(CT)]
            wk_sb = [bigp.tile([P, C], BF16, name=f"wk{t}", tag=f"wk{t}") for t in range(CCT)]
            wv_sb = [bigp.tile([P, C], BF16, name=f"wv{t}", tag=f"wv{t}") for t in range(CCT)]
            wo_sb = [bigp.tile([P, C], BF16, name=f"wo{t}", tag=f"wo{t}") for t in range(CT)]
            # split input loads across the two HWDGE queues (SP + ACT),
            # ordered by first use: Q path (x,wq) on sync, K path (y,wk) on
            # ACT's queue (idle until the first exp); biases and wo later
            for t in range(CT):
                nc.sync.dma_start(x_sb[t][:], x3[:, t])
                nc.sync.dma_start(wq_sb[t][:], wq3[:, t])
            for t in range(CCT):
                nc.scalar.dma_start(y_sb[t][:], y3[:, t])
                nc.scalar.dma_start(wk_sb[t][:], wk3[:, t])
            nc.sync.dma_start(bq_sb[:], bq.rearrange("(o p) -> p o", p=P))
            nc.sync.dma_start(bk_sb[:], bk.rearrange("(o p) -> p o", p=P))
            # junk exp here: the ACT table load lands right after y/wk,
            # before the first real exp needs it
            junk_et = constp.tile([P, DH], BF16, name="junk_et", tag="junk_et")
            nc.scalar.activation(junk_et[:], junk_bf[:, 0:DH], EXP, scale=0.125)
            for t in range(CCT):
                nc.sync.dma_start(wv_sb[t][:], wv3[:, t])
            nc.sync.dma_start(bv_sb[:], bv[None, :])
            for t in range(CT):
                nc.scalar.dma_start(wo_sb[t][:], wo3[:, t])
            nc.sync.dma_start(bo_sb[:], bo.rearrange("(o p) -> p o", p=P))

            q_sb = [bigp.tile([P, N], BF16, name=f"q{t}", tag=f"q{t}") for t in range(CT)]
            k_sb = [bigp.tile([P, N], BF16, name=f"k{t}", tag=f"k{t}") for t in range(CT)]
            # VT buffer: per n-tile, cols laid out [h][65] with col h*65+64 == 1.0
            vt_sb = [bigp.tile([P, NH * (DH + 1)], BF16, name=f"vt{t}", tag=f"vt{t}")
                     for t in range(QT)]
            for t in range(QT):
                nc.gpsimd.memset(vt_sb[t][:], 1.0)

            qk_sb = [bigp.tile([P, N], BF16, name=f"qk{t}", tag=f"qk{t}")
                     for t in range(CT)]
            # ET tiles: double-buffered across pair parity
            et_t = [[[etp.tile([P, N], BF16, name=f"et{par}_{hh}_{kt}",
                               tag=f"et{par}_{hh}_{kt}")
                      for kt in range(QT)] for hh in range(2)] for par in range(2)]
            oa_t = [bigp.tile([P, QT, 2 * DH], BF16, name=f"oa{par}", tag=f"oa{par}")
                    for par in range(2)]

            # ---- projection helpers ----
            def qk_proj(ct, dst, w_tiles, src_tiles, nkt, bias_sb):
                for half in range(2):
                    ps = psP.tile([P, 512], F32, name="ps", tag="psp")
                    for kt in range(nkt):
                        nc.tensor.matmul(
                            ps[:],
                            w_tiles[kt][:, ct * P:(ct + 1) * P],
                            src_tiles[kt][:, half * 512:(half + 1) * 512],
                            start=(kt == 0),
                            stop=(kt == nkt - 1),
                        )
                    nc.vector.tensor_scalar_add(
                        dst[:, half * 512:(half + 1) * 512], ps[:], bias_sb[:, ct:ct + 1]
                    )

            def vt_proj(nt):
                ps = psP.tile([P, 512], F32, name="ps", tag="psp")
                for kt in range(CCT):
                    nc.tensor.matmul(
                        ps[:],
                        y_sb[kt][:, nt * P:(nt + 1) * P],
                        wv_sb[kt][:],
                        start=(kt == 0),
                        stop=False,
                    )
                nc.tensor.matmul(ps[:], ones_bf[:], bv_sb[:], start=False, stop=True)
                # scatter into [h][0:64] slots (col h*65+64 stays 1.0)
                nc.vector.tensor_copy(
                    out=vt_sb[nt].rearrange("p (h e) -> p h e", e=DH + 1)[:, :, 0:DH],
                    in_=ps.rearrange("p (h d) -> p h d", d=DH),
                )

            # ---- deferred-work chunks for a completed pair ----
            # AV accumulation bursts, OT transposes + normalize + quirk DMA.
            COPY = mybir.ActivationFunctionType.Copy

            def av_burst(ph, hh, qh, ot_dst, use_act=False):
                par = ph % 2
                g = (2 * ph + hh) * (DH + 1)
                otps = psX.tile([DH + 1, 512], F32, name="otp", tag="otp")
                for kt in range(QT):
                    nc.tensor.matmul(
                        otps[:],
                        vt_sb[kt][:, g:g + DH + 1],
                        et_t[par][hh][kt][:, qh * 512:(qh + 1) * 512],
                        start=(kt == 0),
                        stop=(kt == QT - 1),
                    )
                if use_act:  # last pair: ScalarE is idle, offload the copy
                    nc.scalar.activation(ot_dst[:], otps[:], COPY)
                else:
                    nc.vector.tensor_copy(out=ot_dst[:], in_=otps[:])

            def tail_chunk(ph, hh, qh, ot_src, use_act=False):
                par = ph % 2
                oa = oa_t[par]
                for qq in range(4):
                    qt = qh * 4 + qq
                    tps = psX.tile([P, DH + 1], F32, name="tps", tag="otp")
                    nc.tensor.transpose(
                        tps[:],
                        ot_src[:, qq * P:(qq + 1) * P],
                        ident[0:DH + 1, 0:DH + 1],
                    )
                    rcp = rcpp.tile([P, 1], F32, name="rcp", tag="rcp")
                    nc.vector.reciprocal(rcp[:], tps[:, DH:DH + 1])
                    if use_act:  # last pair: normalize on the idle ScalarE
                        nc.scalar.activation(
                            oa[:, qt, hh * DH:(hh + 1) * DH], tps[:, 0:DH],
                            COPY, scale=rcp[:],
                        )
                    else:
                        nc.vector.tensor_scalar_mul(
                            oa[:, qt, hh * DH:(hh + 1) * DH], tps[:, 0:DH], rcp[:]
                        )
                    # quirk shuffle: qk[ph*128 + hh*64 + 8*qt + rr, j*64+d]
                    #   = O[128*qt + 16*rr + j, (2ph+hh)*64 + d]
                    eng = nc.scalar if (use_act and qq % 2) else nc.sync
                    eng.dma_start(
                        qk_sb[ph][64 * hh + 8 * qt: 64 * hh + 8 * qt + 8, :],
                        oa[:, qt, hh * DH:(hh + 1) * DH],
                    )

            def pair_deferred_chunks(ph, use_act=False):
                """Chunks of PE-filler work retiring pair ph. AV bursts are
                staggered one ahead of their tails so each tail's transposes
                never wait on the copy of its own burst."""
                avs, tails = [], []
                for hh in range(2):
                    for qh in range(2):
                        ot_sb = otp.tile([DH + 1, 512], F32, name="otsb", tag="otsb")
                        avs.append(
                            (lambda ph=ph, hh=hh, qh=qh, o=ot_sb:
                             av_burst(ph, hh, qh, o, use_act))
                        )
                        tails.append(
                            (lambda ph=ph, hh=hh, qh=qh, o=ot_sb:
                             tail_chunk(ph, hh, qh, o, use_act))
                        )
                return avs, tails

            # ---- attention phase1 (scores+exp), with PE-filler interleave ----
            def phase1(ph, fillers, pad_per_slot=0):
                par = ph % 2
                fi = 0
                nf = len(fillers)
                for kt in range(QT):
                    if pad_per_slot:
                        pad(pad_per_slot)
                    sps = {hh: psS.tile([P, N], F32, name="sps", tag="sps")
                           for hh in range(2)}
                    for half in range(2):
                        for hh in range(2):  # alternate row groups for concurrency
                            bp = hh * DH
                            nc.tensor.matmul(
                                sps[hh][:, half * 512:(half + 1) * 512],
                                k_sb[ph][bp:bp + DH, kt * P:(kt + 1) * P],
                                q_sb[ph][bp:bp + DH, half * 512:(half + 1) * 512],
                                start=True,
                                stop=True,
                            )
                    for hh in range(2):
                        nc.scalar.activation(
                            et_t[par][hh][kt][:], sps[hh][:], EXP, scale=0.125,
                        )
                    # interleave deferred/filler chunks evenly across kt slots
                    want = (kt + 1) * nf // QT
                    while fi < want:
                        fillers[fi]()
                        fi += 1
                while fi < nf:
                    fillers[fi]()
                    fi += 1

            # ---- main schedule ----
            qk_proj(0, q_sb[0], wq_sb, x_sb, CT, bq_sb)
            qk_proj(0, k_sb[0], wk_sb, y_sb, CCT, bk_sb)

            for ph in range(NH // 2):
                if ph == 0:
                    fillers = [lambda nt=nt: vt_proj(nt) for nt in range(QT)]
                    fillers.insert(2, lambda: qk_proj(1, q_sb[1], wq_sb, x_sb, CT, bq_sb))
                    fillers.insert(5, lambda: qk_proj(1, k_sb[1], wk_sb, y_sb, CCT, bk_sb))
                else:
                    avs, tails = pair_deferred_chunks(ph - 1)
                    if ph + 1 < NH // 2:
                        pq = lambda p=ph: qk_proj(p + 1, q_sb[p + 1], wq_sb, x_sb, CT, bq_sb)
                        pk = lambda p=ph: qk_proj(p + 1, k_sb[p + 1], wk_sb, y_sb, CCT, bk_sb)
                        # next-pair projections early so phase1(ph+1) never
                        # stalls on Q/K; av bursts early so et WARs clear
                        fillers = [avs[0], pq, avs[1], tails[0], avs[2], pk,
                                   tails[1], avs[3], tails[2], tails[3]]
                    else:
                        fillers = [avs[0], avs[1], tails[0], avs[2], tails[1],
                                   avs[3], tails[2], tails[3]]
                phase1(ph, fillers, pad_per_slot=(0 if ph < 2 else (4 if ph == 2 else 10)))

            # retire the last pair on the now-idle ScalarE (padded to keep
            # the HAM gate warm)
            avs, tails = pair_deferred_chunks(NH // 2 - 1, use_act=True)
            for ch in [avs[0], avs[1], tails[0], avs[2], tails[1], avs[3],
                       tails[2], tails[3]]:
                pad(8)
                ch()

            # ---- output projection ----
            # (kt ascending: only the kt=3 matmul waits on pair-3's quirk
            # DMAs, so kt 0-2 of the first groups run during the tail)
            out3 = out.rearrange("(t p) n -> p t n", p=P)
            for ct in range(CT):
                for half in range(2):
                    ps = psP.tile([P, 512], F32, name="ps", tag="psp")
                    for kt in range(CT):
                        nc.tensor.matmul(
                            ps[:],
                            wo_sb[kt][:, ct * P:(ct + 1) * P],
                            qk_sb[kt][:, half * 512:(half + 1) * 512],
                            start=(kt == 0),
                            stop=(kt == CT - 1),
                        )
                    ev = evp.tile([P, 512], F32, name="ev", tag="ev")
                    nc.vector.tensor_scalar_add(ev[:], ps[:], bo_sb[:, ct:ct + 1])
                    eng = nc.sync if (ct + half) % 2 == 0 else nc.scalar
                    eng.dma_start(out3[:, ct, half * 512:(half + 1) * 512], ev[:])
                    pad(4)

    nc.compile()
    return nc


def kernel(**inputs) -> np.ndarray:
    import ml_dtypes
    bf = ml_dtypes.bfloat16

    x = np.ascontiguousarray(np.asarray(inputs["x"], dtype=np.float32).astype(bf))
    y = np.ascontiguousarray(np.asarray(inputs["y"], dtype=np.float32).astype(bf))
    wqT = np.ascontiguousarray(np.asarray(inputs["w_q"], dtype=np.float32).T.astype(bf))
    wkT = np.ascontiguousarray(np.asarray(inputs["w_k"], dtype=np.float32).T.astype(bf))
    wvT = np.ascontiguousarray(np.asarray(inputs["w_v"], dtype=np.float32).T.astype(bf))
    woT = np.ascontiguousarray(np.asarray(inputs["w_o"], dtype=np.float32).T.astype(bf))
    bq = np.ascontiguousarray(np.asarray(inputs["b_q"], dtype=np.float32))
    bk = np.ascontiguousarray(np.asarray(inputs["b_k"], dtype=np.float32))
    bv = np.ascontiguousarray(np.asarray(inputs["b_v"], dtype=np.float32).astype(bf))
    bo = np.ascontiguousarray(np.asarray(inputs["b_o"], dtype=np.float32))

    if "nc" not in _CACHE:
        _CACHE["nc"] = _build_nc()
    nc = _CACHE["nc"]

    in_maps = []
    for b in range(B):
        in_maps.append({
            "x": np.ascontiguousarray(x[b].reshape(C, N)),
            "y": np.ascontiguousarray(y[b].reshape(CC, N)),
            "wqT": wqT, "wkT": wkT, "wvT": wvT, "woT": woT,
            "bq": bq, "bk": bk, "bv": bv, "bo": bo,
        })
    res = run_bass_kernel_spmd(nc, in_maps, core_ids=list(range(B)))
    return np.stack([res.results[b]["out"].reshape(C, HW, HW) for b in range(B)])


# revision 27
# speedup vs baseline: 1.1323x; 1.1323x over previous
"""CrossAttention2D Trainium2 kernel (bf16 compute).

Sharding: data-parallel over batch. B=8 -> one batch element per NeuronCore,
no collectives. Weights replicated; host pre-transposes and casts to bf16.

Per-core math (C=512, Ccross=768, N=1024, 8 heads x 64):
  Q = Wq @ x_b          [C, N]   (lhsT = WqT tiles, bf16)
  K = Wk @ y_b          [C, N]
  VT = (Wv @ y_b).T     [N, C]   (lhsT = y tiles, rhs = WvT; bias via K=1 matmul)
  per head pair ph (heads on PE rows 0-63 / 64-127, row-tiled concurrent MMs):
    ST = K_h^T . Q_h              [k, q] tiles (K=64 matmuls)
    ET = exp(0.125 * ST)          (ScalarE, free=1024 per instr, PSUM -> SBUF bf16)
    OT_aug = [VT_h | 1].T @ ET    [65, q] in [65,512] accumulation groups
    PE-transpose -> [q, 65]; DVE: O = OT[:, :64] * (1/OT[:, 64])
  quirk: out_flat[h*64+r, j*64+d] = O_h[16r+j, d]  (SBUF->SBUF DMAs)
  out = Wo @ quirk + bo  [C, N] fp32

Schedule: the scores+exp loop is ScalarE-bound; AV bursts + transposes +
tail of the previous pair and projections of the next pair are interleaved
into the PE stream to keep the HAM clock-gate warm. A dummy-matmul warmup
runs during the input-DMA head. PSUM: psS 2x[128,1024] (4 banks) +
psX 4x[128,512]-class shared by AV/transpose/proj (4 banks) = 8.
"""

import numpy as np

import concourse.bass as bass
import concourse.mybir as mybir
import concourse.tile as tile
from concourse import bacc
from concourse.bass_utils import run_bass_kernel_spmd
from concourse.masks import make_identity

P = 128
C = 512          # d_embed
CC = 768         # d_cross
N = 1024         # H*W = 32*32
NH = 8
DH = 64
CT = C // P      # 4
CCT = CC // P    # 6
QT = N // P      # 8
HW = 32
B = 8
F32 = mybir.dt.float32
BF16 = mybir.dt.bfloat16

_CACHE = {}


def _build_nc():
    nc = bacc.Bacc("TRN2", target_bir_lowering=False, debug=False, num_devices=B)

    x = nc.dram_tensor("x", [C, N], BF16, kind="ExternalInput")
    y = nc.dram_tensor("y", [CC, N], BF16, kind="ExternalInput")
    wqT = nc.dram_tensor("wqT", [C, C], BF16, kind="ExternalInput")
    wkT = nc.dram_tensor("wkT", [CC, C], BF16, kind="ExternalInput")
    wvT = nc.dram_tensor("wvT", [CC, C], BF16, kind="ExternalInput")
    woT = nc.dram_tensor("woT", [C, C], BF16, kind="ExternalInput")
    bq = nc.dram_tensor("bq", [C], F32, kind="ExternalInput")
    bk = nc.dram_tensor("bk", [C], F32, kind="ExternalInput")
    bv = nc.dram_tensor("bv", [C], BF16, kind="ExternalInput")
    bo = nc.dram_tensor("bo", [C], F32, kind="ExternalInput")
    out = nc.dram_tensor("out", [C, N], F32, kind="ExternalOutput")

    EXP = mybir.ActivationFunctionType.Exp

    with tile.TileContext(nc) as tc:
        with (
            tc.tile_pool(name="const", bufs=1) as constp,
            tc.tile_pool(name="big", bufs=1) as bigp,
            tc.tile_pool(name="et", bufs=1) as etp,
            tc.tile_pool(name="ot", bufs=4) as otp,
            tc.tile_pool(name="rcp", bufs=4) as rcpp,
            tc.tile_pool(name="ev", bufs=4) as evp,
            tc.tile_pool(name="psS", bufs=2, space="PSUM") as psS,
            tc.tile_pool(name="psP", bufs=2, space="PSUM") as psP,
            tc.tile_pool(name="psX", bufs=2, space="PSUM") as psX,
        ):
            # ---- constants ----
            junk_bf = constp.tile([P, P], BF16, name="junk", tag="junk")
            nc.gpsimd.memset(junk_bf[:], 0.125)
            ident = constp.tile([P, P], F32, name="ident", tag="ident")
            make_identity(nc, ident)
            ones_bf = constp.tile([1, P], BF16, name="ones_bf", tag="ones_bf")
            nc.vector.tensor_scalar(
                ones_bf[:], ident[0:1, :], 0.0, 1.0,
                mybir.AluOpType.mult, mybir.AluOpType.add,
            )
            bq_sb = constp.tile([P, CT], F32, name="bq", tag="bq")
            bk_sb = constp.tile([P, CT], F32, name="bk", tag="bk")
            bo_sb = constp.tile([P, CT], F32, name="bo", tag="bo")
            bv_sb = constp.tile([1, C], BF16, name="bv", tag="bv")

            # ---- PE + ACT warmup (runs while input DMAs land) ----
            # ~36 dummy matmuls keep the PE busy >3.4us so the HAM clock
            # gate reaches 8/8 before the first real projection; one junk
            # exp pulls the ACT table load off the critical path.
            psw = psP.tile([P, P], F32, name="psw", tag="psp")
            for _ in range(40):
                nc.tensor.matmul(psw[:], junk_bf[:], junk_bf[:],
                                 start=True, stop=True)

            def pad(n):
                # dependency-free LDWEIGHTS keep the PE HAM activity monitor
                # busy through ScalarE-bound stretches (junk loads are safe:
                # every real matmul self-loads its own weights)
                for _ in range(n):
                    nc.tensor.ldweights(junk_bf[:])

            # ---- weight / activation loads (per-ktile DMAs for queue spread) ----
            x3 = x.rearrange("(t p) n -> p t n", p=P)
            y3 = y.rearrange("(t p) n -> p t n", p=P)
            wq3 = wqT.rearrange("(t p) m -> p t m", p=P)
            wk3 = wkT.rearrange("(t p) m -> p t m", p=P)
            wv3 = wvT.rearrange("(t p) m -> p t m", p=P)
            wo3 = woT.rearrange("(t p) m -> p t m", p=P)

            x_sb = [bigp.tile([P, N], BF16, name=f"x{t}", tag=f"x{t}") for t in range(CT)]
            y_sb = [bigp.tile([P, N], BF16, name=f"y{t}", tag=f"y{t}") for t in range(CCT)]
            wq_sb = [bigp.tile([P, C], BF16, name=f"wq{t}", tag=f"wq{t}") for t in range(CT)]
            wk_sb = [bigp.tile([P, C], BF16, name=f"wk{t}", tag=f"wk{t}") for t in range(CCT)]
            wv_sb = [bigp.tile([P, C], BF16, name=f"wv{t}", tag=f"wv{t}") for t in range(CCT)]
            wo_sb = [bigp.tile([P, C], BF16, name=f"wo{t}", tag=f"wo{t}") for t in range(CT)]
            # stripe the head loads across both HWDGE queues (SP + ACT) in
            # need-order so x/wq, then y/wk, stream on both queues at once.
            # Everything needed after the first exp stays OFF the ACT queue
            # (its instruction stream must be free for the exp chain).
            nc.sync.dma_start(bq_sb[:], bq.rearrange("(o p) -> p o", p=P))
            nc.sync.dma_start(bk_sb[:], bk.rearrange("(o p) -> p o", p=P))
            qs = [nc.sync, nc.scalar]
            for t in range(CT):
                qs[t % 2].dma_start(x_sb[t][:], x3[:, t])
                qs[t % 2].dma_start(wq_sb[t][:], wq3[:, t])
            for t in range(CCT):
                qs[t % 2].dma_start(y_sb[t][:], y3[:, t])
                qs[t % 2].dma_start(wk_sb[t][:], wk3[:, t])
            # junk exp: ACT table load lands right after y/wk, before the
            # first real exp needs it
            junk_et = constp.tile([P, DH], BF16, name="junk_et", tag="junk_et")
            nc.scalar.activation(junk_et[:], junk_bf[:, 0:DH], EXP, scale=0.125)
            for t in range(CCT):
                nc.sync.dma_start(wv_sb[t][:], wv3[:, t])
            nc.sync.dma_start(bv_sb[:], bv[None, :])
            for t in range(CT):
                nc.sync.dma_start(wo_sb[t][:], wo3[:, t])
            nc.sync.dma_start(bo_sb[:], bo.rearrange("(o p) -> p o", p=P))

            q_sb = [bigp.tile([P, N], BF16, name=f"q{t}", tag=f"q{t}") for t in range(CT)]
            k_sb = [bigp.tile([P, N], BF16, name=f"k{t}", tag=f"k{t}") for t in range(CT)]
            # VT buffer: per n-tile, cols laid out [h][65] with col h*65+64 == 1.0
            vt_sb = [bigp.tile([P, NH * (DH + 1)], BF16, name=f"vt{t}", tag=f"vt{t}")
                     for t in range(QT)]
            for t in range(QT):
                nc.gpsimd.memset(vt_sb[t][:], 1.0)

            qk_sb = [bigp.tile([P, N], BF16, name=f"qk{t}", tag=f"qk{t}")
                     for t in range(CT)]
            # ET tiles: double-buffered across pair parity
            et_t = [[[etp.tile([P, N], BF16, name=f"et{par}_{hh}_{kt}",
                               tag=f"et{par}_{hh}_{kt}")
                      for kt in range(QT)] for hh in range(2)] for par in range(2)]
            oa_t = [bigp.tile([P, QT, 2 * DH], BF16, name=f"oa{par}", tag=f"oa{par}")
                    for par in range(2)]

            # ---- projection helpers ----
            def qk_proj(ct, dst, w_tiles, src_tiles, nkt, bias_sb):
                for half in range(2):
                    ps = psP.tile([P, 512], F32, name="ps", tag="psp")
                    for kt in range(nkt):
                        nc.tensor.matmul(
                            ps[:],
                            w_tiles[kt][:, ct * P:(ct + 1) * P],
                            src_tiles[kt][:, half * 512:(half + 1) * 512],
                            start=(kt == 0),
                            stop=(kt == nkt - 1),
                        )
                    nc.vector.tensor_scalar_add(
                        dst[:, half * 512:(half + 1) * 512], ps[:], bias_sb[:, ct:ct + 1]
                    )

            def vt_proj(nt):
                ps = psP.tile([P, 512], F32, name="ps", tag="psp")
                for kt in range(CCT):
                    nc.tensor.matmul(
                        ps[:],
                        y_sb[kt][:, nt * P:(nt + 1) * P],
                        wv_sb[kt][:],
                        start=(kt == 0),
                        stop=False,
                    )
                nc.tensor.matmul(ps[:], ones_bf[:], bv_sb[:], start=False, stop=True)
                # scatter into [h][0:64] slots (col h*65+64 stays 1.0)
                nc.vector.tensor_copy(
                    out=vt_sb[nt].rearrange("p (h e) -> p h e", e=DH + 1)[:, :, 0:DH],
                    in_=ps.rearrange("p (h d) -> p h d", d=DH),
                )

            # ---- deferred-work chunks for a completed pair ----
            # AV accumulation bursts, OT transposes + normalize + quirk DMA.
            COPY = mybir.ActivationFunctionType.Copy

            def av_burst(ph, hh, qh, ot_dst, use_act=False):
                par = ph % 2
                g = (2 * ph + hh) * (DH + 1)
                otps = psX.tile([DH + 1, 512], F32, name="otp", tag="otp")
                for kt in range(QT):
                    nc.tensor.matmul(
                        otps[:],
                        vt_sb[kt][:, g:g + DH + 1],
                        et_t[par][hh][kt][:, qh * 512:(qh + 1) * 512],
                        start=(kt == 0),
                        stop=(kt == QT - 1),
                    )
                if use_act and (hh + qh) % 2:  # last pair: split copies DVE/ACT
                    nc.scalar.activation(ot_dst[:], otps[:], COPY)
                else:
                    nc.vector.tensor_copy(out=ot_dst[:], in_=otps[:])

            def tail_chunk(ph, hh, qh, ot_src, use_act=False):
                par = ph % 2
                oa = oa_t[par]
                for qq in range(4):
                    qt = qh * 4 + qq
                    tps = psP.tile([P, DH + 1], F32, name="tps", tag="psp")
                    nc.tensor.transpose(
                        tps[:],
                        ot_src[:, qq * P:(qq + 1) * P],
                        ident[0:DH + 1, 0:DH + 1],
                    )
                    rcp = rcpp.tile([P, 1], F32, name="rcp", tag="rcp")
                    nc.vector.reciprocal(rcp[:], tps[:, DH:DH + 1])
                    if use_act and qq % 2:  # last pair: split normalize DVE/ACT
                        nc.scalar.activation(
                            oa[:, qt, hh * DH:(hh + 1) * DH], tps[:, 0:DH],
                            COPY, scale=rcp[:],
                        )
                    else:
                        nc.vector.tensor_scalar_mul(
                            oa[:, qt, hh * DH:(hh + 1) * DH], tps[:, 0:DH], rcp[:]
                        )
                    # quirk shuffle: qk[ph*128 + hh*64 + 8*qt + rr, j*64+d]
                    #   = O[128*qt + 16*rr + j, (2ph+hh)*64 + d]
                    nc.sync.dma_start(
                        qk_sb[ph][64 * hh + 8 * qt: 64 * hh + 8 * qt + 8, :],
                        oa[:, qt, hh * DH:(hh + 1) * DH],
                    )

            def pair_deferred_chunks(ph, use_act=False):
                """Chunks of PE-filler work retiring pair ph. AV bursts are
                staggered one ahead of their tails so each tail's transposes
                never wait on the copy of its own burst."""
                avs, tails = [], []
                for hh in range(2):
                    for qh in range(2):
                        ot_sb = otp.tile([DH + 1, 512], F32, name="otsb", tag="otsb")
                        avs.append(
                            (lambda ph=ph, hh=hh, qh=qh, o=ot_sb:
                             av_burst(ph, hh, qh, o, use_act))
                        )
                        tails.append(
                            (lambda ph=ph, hh=hh, qh=qh, o=ot_sb:
                             tail_chunk(ph, hh, qh, o, use_act))
                        )
                return avs, tails

            # ---- attention phase1 (scores+exp), with PE-filler interleave ----
            def phase1(ph, fillers, pad_per_slot=0):
                par = ph % 2
                fi = 0
                nf = len(fillers)
                for kt in range(QT):
                    if pad_per_slot:
                        pad(pad_per_slot)
                    sps = {hh: psS.tile([P, N], F32, name="sps", tag="sps")
                           for hh in range(2)}
                    for half in range(2):
                        for hh in range(2):  # alternate row groups for concurrency
                            bp = hh * DH
                            nc.tensor.matmul(
                                sps[hh][:, half * 512:(half + 1) * 512],
                                k_sb[ph][bp:bp + DH, kt * P:(kt + 1) * P],
                                q_sb[ph][bp:bp + DH, half * 512:(half + 1) * 512],
                                start=True,
                                stop=True,
                            )
                    for hh in range(2):
                        nc.scalar.activation(
                            et_t[par][hh][kt][:], sps[hh][:], EXP, scale=0.125,
                        )
                    # interleave deferred/filler chunks evenly across kt slots
                    want = (kt + 1) * nf // QT
                    while fi < want:
                        fillers[fi]()
                        fi += 1
                while fi < nf:
                    fillers[fi]()
                    fi += 1

            # ---- main schedule ----
            qk_proj(0, q_sb[0], wq_sb, x_sb, CT, bq_sb)
            qk_proj(0, k_sb[0], wk_sb, y_sb, CCT, bk_sb)

            for ph in range(NH // 2):
                if ph == 0:
                    fillers = [lambda nt=nt: vt_proj(nt) for nt in range(QT)]
                    fillers.insert(2, lambda: qk_proj(1, q_sb[1], wq_sb, x_sb, CT, bq_sb))
                    fillers.insert(5, lambda: qk_proj(1, k_sb[1], wk_sb, y_sb, CCT, bk_sb))
                else:
                    avs, tails = pair_deferred_chunks(ph - 1)
                    if ph + 1 < NH // 2:
                        pq = lambda p=ph: qk_proj(p + 1, q_sb[p + 1], wq_sb, x_sb, CT, bq_sb)
                        pk = lambda p=ph: qk_proj(p + 1, k_sb[p + 1], wk_sb, y_sb, CCT, bk_sb)
                        # next-pair projections mid-phase so phase1(ph+1)
                        # never stalls on Q/K; av bursts early so et WARs
                        # clear; tails spread through the middle
                        fillers = [avs[0], avs[1], pq, tails[0], avs[2], pk,
                                   tails[1], avs[3], tails[2], tails[3]]
                    else:
                        fillers = [avs[0], avs[1], tails[0], avs[2], tails[1],
                                   avs[3], tails[2], tails[3]]
                phase1(ph, fillers, pad_per_slot=(2 if ph < 2 else (4 if ph == 2 else 10)))

            # retire the last pair on the now-idle ScalarE (padded to keep
            # the HAM gate warm)
            avs, tails = pair_deferred_chunks(NH // 2 - 1, use_act=True)
            for ch in [avs[0], avs[1], tails[0], avs[2], tails[1], avs[3],
                       tails[2], tails[3]]:
                pad(8)
                ch()

            # ---- output projection ----
            # (kt ascending: only the kt=3 matmul waits on pair-3's quirk
            # DMAs, so kt 0-2 of the first groups run during the tail)
            out3 = out.rearrange("(t p) n -> p t n", p=P)
            for ct in range(CT):
                for half in range(2):
                    ps = psP.tile([P, 512], F32, name="ps", tag="psp")
                    for kt in range(CT):
                        nc.tensor.matmul(
                            ps[:],
                            wo_sb[kt][:, ct * P:(ct + 1) * P],
                            qk_sb[kt][:, half * 512:(half + 1) * 512],
                            start=(kt == 0),
                            stop=(kt == CT - 1),
                        )
                    ev = evp.tile([P, 512], F32, name="ev", tag="ev")
                    nc.vector.tensor_scalar_add(ev[:], ps[:], bo_sb[:, ct:ct + 1])
                    eng = nc.sync if (ct + half) % 2 == 0 else nc.scalar
                    eng.dma_start(out3[:, ct, half * 512:(half + 1) * 512], ev[:])
                    pad(4)

    nc.compile()
    return nc


def kernel(**inputs) -> np.ndarray:
    import ml_dtypes
    bf = ml_dtypes.bfloat16

    x = np.ascontiguousarray(np.asarray(inputs["x"], dtype=np.float32).astype(bf))
    y = np.ascontiguousarray(np.asarray(inputs["y"], dtype=np.float32).astype(bf))
    wqT = np.ascontiguousarray(np.asarray(inputs["w_q"], dtype=np.float32).T.astype(bf))
    wkT = np.ascontiguousarray(np.asarray(inputs["w_k"], dtype=np.float32).T.astype(bf))
    wvT = np.ascontiguousarray(np.asarray(inputs["w_v"], dtype=np.float32).T.astype(bf))
    woT = np.ascontiguousarray(np.asarray(inputs["w_o"], dtype=np.float32).T.astype(bf))
    bq = np.ascontiguousarray(np.asarray(inputs["b_q"], dtype=np.float32))
    bk = np.ascontiguousarray(np.asarray(inputs["b_k"], dtype=np.float32))
    bv = np.ascontiguousarray(np.asarray(inputs["b_v"], dtype=np.float32).astype(bf))
    bo = np.ascontiguousarray(np.asarray(inputs["b_o"], dtype=np.float32))

    if "nc" not in _CACHE:
        _CACHE["nc"] = _build_nc()
    nc = _CACHE["nc"]

    in_maps = []
    for b in range(B):
        in_maps.append({
            "x": np.ascontiguousarray(x[b].reshape(C, N)),
            "y": np.ascontiguousarray(y[b].reshape(CC, N)),
            "wqT": wqT, "wkT": wkT, "wvT": wvT, "woT": woT,
            "bq": bq, "bk": bk, "bv": bv, "bo": bo,
        })
    res = run_bass_kernel_spmd(nc, in_maps, core_ids=list(range(B)))
    return np.stack([res.results[b]["out"].reshape(C, HW, HW) for b in range(B)])
